# revision 6
# baseline (speedup 1.0000x reference)
"""LocallyConnected2D (B=16, H=W=64, C=32, 3x3 valid, F=64) on 8 trn2 cores.

out[b, oh, ow, f] = sum_{kh,kw,c} x[b, oh+kh, ow+kw, c] * kernel[p, (kh,kw,c), f] + bias[p, f]
with p = oh*62+ow.  P=3844 sharded by oh-rows across 8 cores (8 rows/core,
core 7 padded).

Per core: weights stream from HBM in fp16 (one DMA per oh-row, [97 x 11904]
row-major, partition row 96 = bias folded into the kh=0 chunk); patches are
pre-transposed on the host into [97 x 992] fp16 tiles (row 96 = ones) so no
on-device im2col or transposes are needed.  Each position runs 3 stationary
matmuls ([97,64] weights stationary, 16 batch columns moving) accumulating
into a [64, 992] PSUM row; the row is cast/copied to fp16 SBUF and written
out f-major (host unscrambles the layout).
"""

import sys

for _p in ("/opt/trn_rl_repo",):
    if _p not in sys.path:
        sys.path.insert(0, _p)

import numpy as np
from contextlib import ExitStack

import concourse.bass as bass
import concourse.bacc as bacc
import concourse.mybir as mybir
import concourse.tile as tile
from concourse.bass_utils import run_bass_kernel_spmd

F32 = mybir.dt.float32
F16 = mybir.dt.float16

B, H, W, C = 16, 64, 64, 32
KH, KW = 3, 3
OH, OW = 62, 62
F = 64
NCORES = 8
RPC = 8            # oh rows per core (core 7: 2 rows are padding)
NXR = RPC + 2      # x rows staged per core
PPC = RPC * OW     # 496 positions per core (padded for core 7)
KP = KW * C + 1    # 97 partitions: 96 contraction rows + bias/ones row
HOW = OW // 2      # 31 positions per half-row (pipeline granule)
WHROW = HOW * KH * F  # 5952 free elements per weight half-row tile

_cached = {}


def _build_program():
    if "nc" in _cached:
        return _cached["nc"]

    nc = bacc.Bacc(None)
    # xt[r, kw*32+c, ow*16+b] = x[b, r0+r, ow+kw, c]; row 96 = 1.0
    xt = nc.declare_dram_parameter("xt", [NXR, KP, OW * B], F16, isOutput=False)
    # ks[hr, kw*32+c, (ow'*3+ch)*64+f] = kernel[p, ch*96+kw*32+c, f] with
    # oh = hr//2, ow = (hr%2)*31 + ow';
    # row 96: bias[p, f] at ch==0, zero at ch 1..2
    ks = nc.declare_dram_parameter("ks", [2 * RPC, KP, WHROW], F16, isOutput=False)
    # out[hr, f, ow'*16+b]
    out = nc.declare_dram_parameter("out", [2 * RPC, F, HOW * B], F16, isOutput=True)

    with ExitStack() as ctx:
        tc = ctx.enter_context(tile.TileContext(nc))
        tpool = ctx.enter_context(tc.tile_pool(name="tpool", bufs=NXR))
        ktpool = ctx.enter_context(tc.tile_pool(name="ktpool", bufs=4))
        pspool = ctx.enter_context(tc.tile_pool(name="pspool", bufs=4, space="PSUM"))
        stpool = ctx.enter_context(tc.tile_pool(name="stpool", bufs=4))

        # x patch tiles: rows 0..2 first (row 0 of compute needs them), rest
        # prefetched during the row loop so kt[0] isn't delayed.
        T = []
        for r in range(NXR):
            t_tile = tpool.tile([KP, OW * B], F16)
            T.append(t_tile)
        for r in range(KH):
            nc.sync.dma_start(T[r][:, :], xt[r])

        for hr in range(2 * RPC):
            oh, half = divmod(hr, 2)
            ow0 = half * HOW
            kt = ktpool.tile([KP, WHROW], F16)
            nc.sync.dma_start(kt[:, :], ks[hr])
            if half == 0 and oh + KH < NXR:
                r = oh + KH
                nc.sync.dma_start(T[r][:, :], xt[r])
            ps = pspool.tile([F, HOW * B], F32)
            for owl in range(HOW):
                ow = ow0 + owl
                for ch in range(KH):
                    nc.tensor.matmul(
                        ps[0:F, owl * B : (owl + 1) * B],
                        kt[0:KP, (owl * KH + ch) * F : (owl * KH + ch + 1) * F],
                        T[oh + ch][0:KP, ow * B : (ow + 1) * B],
                        start=(ch == 0),
                        stop=(ch == KH - 1),
                    )
            st = stpool.tile([F, HOW * B], F16)
            nc.vector.tensor_copy(st[:, :], ps[:, :])
            nc.sync.dma_start(out[hr], st[:, :])

    nc.finalize()
    _cached["nc"] = nc
    return nc


def _shard_inputs(x, kernel, bias):
    x = np.asarray(x, dtype=np.float32)
    kernel = np.asarray(kernel, dtype=np.float32)
    bias = np.asarray(bias, dtype=np.float32)
    kernel16 = kernel.astype(np.float16)   # (P, 288, 64)
    bias16 = bias.astype(np.float16)       # (P, 64)
    x16 = x.astype(np.float16)             # (B, H, W, C)

    in_maps = []
    for c in range(NCORES):
        r0 = RPC * c
        nrows = min(NXR, H - r0)
        xs_c = np.zeros((NXR, B, W, C), dtype=np.float16)
        xs_c[:nrows] = np.moveaxis(x16[:, r0 : r0 + nrows], 1, 0)

        xt_c = np.empty((NXR, KP, OW * B), dtype=np.float16)
        xt_c[:, KP - 1, :] = np.float16(1.0)
        for kw in range(KW):
            # (NXR, B, OW, C) -> (NXR, C, OW, B)
            blk = xs_c[:, :, kw : kw + OW, :].transpose(0, 3, 2, 1)
            xt_c[:, kw * C : (kw + 1) * C, :] = blk.reshape(NXR, C, OW * B)

        ks_c = np.zeros((2 * RPC, KP, WHROW), dtype=np.float16)
        p0 = PPC * c
        pe = min(p0 + PPC, OH * OW)
        nrow_p = (pe - p0) // OW  # full oh rows on this core (8, or 6 on core 7)
        if nrow_p:
            kblk = kernel16[p0 : p0 + nrow_p * OW]  # (nrow*62, 288, 64)
            kblk = kblk.reshape(nrow_p * 2, HOW, KH, KW * C, F)
            # -> (halfrow, kwc, ow', ch, f)
            ks_c[: nrow_p * 2, : KW * C, :] = kblk.transpose(0, 3, 1, 2, 4).reshape(
                nrow_p * 2, KW * C, WHROW
            )
            brow = np.zeros((nrow_p * 2, HOW, KH, F), dtype=np.float16)
            brow[:, :, 0, :] = bias16[p0 : p0 + nrow_p * OW].reshape(
                nrow_p * 2, HOW, F
            )
            ks_c[: nrow_p * 2, KP - 1, :] = brow.reshape(nrow_p * 2, WHROW)

        in_maps.append({"xt": xt_c, "ks": ks_c})
    return in_maps


def _run(x, kernel, bias, trace=False):
    nc = _build_program()
    in_maps = _shard_inputs(x, kernel, bias)
    res = run_bass_kernel_spmd(nc, in_maps, core_ids=list(range(NCORES)), trace=trace)
    out_full = np.empty((B, OH, OW, F), dtype=np.float32)
    for c in range(NCORES):
        rows = min(RPC, OH - RPC * c)
        o = np.asarray(res.results[c]["out"], dtype=np.float32)  # (16, 64, 496)
        # (hr, f, ow', b) -> (b, oh, half*31+ow', f)
        o = o.reshape(RPC, 2, F, HOW, B).transpose(4, 0, 1, 3, 2)
        o = o.reshape(B, RPC, OW, F)
        out_full[:, RPC * c : RPC * c + rows] = o[:, :rows]
    return out_full, res


def kernel(x, kernel, bias):
    out, _ = _run(x, kernel, bias, trace=False)
    return out


# revision 10
# speedup vs baseline: 1.1466x; 1.1466x over previous
"""LocallyConnected2D (B=16, H=W=64, C=32, 3x3 valid, F=64) on 8 trn2 cores.

out[b, oh, ow, f] = sum_{kh,kw,c} x[b, oh+kh, ow+kw, c] * kernel[p, (kh,kw,c), f] + bias[p, f]
with p = oh*62+ow.  P=3844 sharded by oh-rows across 8 cores (8 rows/core,
core 7 padded).

Per core: weights stream from HBM in fp16 (one DMA per oh-row, [97 x 11904]
row-major, partition row 96 = bias folded into the kh=0 chunk); patches are
pre-transposed on the host into [97 x 992] fp16 tiles (row 96 = ones) so no
on-device im2col or transposes are needed.  Each position runs 3 stationary
matmuls ([97,64] weights stationary, 16 batch columns moving) accumulating
into a [64, 992] PSUM row; the row is cast/copied to fp16 SBUF and written
out f-major (host unscrambles the layout).
"""

import sys

for _p in ("/opt/trn_rl_repo",):
    if _p not in sys.path:
        sys.path.insert(0, _p)

import numpy as np
from contextlib import ExitStack

import concourse.bass as bass
import concourse.bacc as bacc
import concourse.mybir as mybir
import concourse.tile as tile
from concourse.bass_utils import run_bass_kernel_spmd

F32 = mybir.dt.float32
F16 = mybir.dt.float16

B, H, W, C = 16, 64, 64, 32
KH, KW = 3, 3
OH, OW = 62, 62
F = 64
NCORES = 8
RPC = 8            # oh rows per core (core 7: 2 rows are padding)
NXR = RPC + 2      # x rows staged per core
PPC = RPC * OW     # 496 positions per core (padded for core 7)
KP = KW * C + 1    # 97 partitions: 96 contraction rows + bias/ones row
HOW = OW // 2      # 31 positions per half-row (pipeline granule)
WHROW = HOW * KH * F  # 5952 free elements per weight half-row tile

_cached = {}


def _build_program():
    if "nc" in _cached:
        return _cached["nc"]

    nc = bacc.Bacc(None)
    # xt[r, kw*32+c, ow*16+b] = x[b, r0+r, ow+kw, c]; row 96 = 1.0
    xt = nc.declare_dram_parameter("xt", [NXR, KP, OW * B], F16, isOutput=False)
    # ks[hr, kw*32+c, (ow'*3+ch)*64+f] = kernel[p, ch*96+kw*32+c, f] with
    # oh = hr//2, ow = (hr%2)*31 + ow';
    # row 96: bias[p, f] at ch==0, zero at ch 1..2
    ks = nc.declare_dram_parameter("ks", [2 * RPC, KP, WHROW], F16, isOutput=False)
    # out[hr, f, ow'*16+b]
    out = nc.declare_dram_parameter("out", [2 * RPC, F, HOW * B], F16, isOutput=True)

    with ExitStack() as ctx:
        tc = ctx.enter_context(tile.TileContext(nc))
        tpool = ctx.enter_context(tc.tile_pool(name="tpool", bufs=NXR))
        ktpool = ctx.enter_context(tc.tile_pool(name="ktpool", bufs=4))
        pspool = ctx.enter_context(tc.tile_pool(name="pspool", bufs=4, space="PSUM"))
        stpool = ctx.enter_context(tc.tile_pool(name="stpool", bufs=4))

        # x patch tiles: rows 0..2 first (row 0 of compute needs them), rest
        # prefetched during the row loop so kt[0] isn't delayed.
        T = []
        for r in range(NXR):
            t_tile = tpool.tile([KP, OW * B], F16)
            T.append(t_tile)
        for r in range(KH):
            nc.scalar.dma_start(T[r][:, :], xt[r])

        for hr in range(2 * RPC):
            oh, half = divmod(hr, 2)
            ow0 = half * HOW
            kt = ktpool.tile([KP, WHROW], F16)
            nc.sync.dma_start(kt[:, :], ks[hr])
            if half == 0 and oh + KH < NXR:
                r = oh + KH
                nc.scalar.dma_start(T[r][:, :], xt[r])
            ps = pspool.tile([F, HOW * B], F32)
            for owl in range(HOW):
                ow = ow0 + owl
                for ch in range(KH):
                    nc.tensor.matmul(
                        ps[0:F, owl * B : (owl + 1) * B],
                        kt[0:KP, (owl * KH + ch) * F : (owl * KH + ch + 1) * F],
                        T[oh + ch][0:KP, ow * B : (ow + 1) * B],
                        start=(ch == 0),
                        stop=(ch == KH - 1),
                    )
            st = stpool.tile([F, HOW * B], F16)
            nc.vector.tensor_copy(st[:, :], ps[:, :])
            nc.scalar.dma_start(out[hr], st[:, :])

    nc.finalize()
    _cached["nc"] = nc
    return nc


def _shard_inputs(x, kernel, bias):
    x = np.asarray(x, dtype=np.float32)
    kernel = np.asarray(kernel, dtype=np.float32)
    bias = np.asarray(bias, dtype=np.float32)
    kernel16 = kernel.astype(np.float16)   # (P, 288, 64)
    bias16 = bias.astype(np.float16)       # (P, 64)
    x16 = x.astype(np.float16)             # (B, H, W, C)

    in_maps = []
    for c in range(NCORES):
        r0 = RPC * c
        nrows = min(NXR, H - r0)
        xs_c = np.zeros((NXR, B, W, C), dtype=np.float16)
        xs_c[:nrows] = np.moveaxis(x16[:, r0 : r0 + nrows], 1, 0)

        xt_c = np.empty((NXR, KP, OW * B), dtype=np.float16)
        xt_c[:, KP - 1, :] = np.float16(1.0)
        for kw in range(KW):
            # (NXR, B, OW, C) -> (NXR, C, OW, B)
            blk = xs_c[:, :, kw : kw + OW, :].transpose(0, 3, 2, 1)
            xt_c[:, kw * C : (kw + 1) * C, :] = blk.reshape(NXR, C, OW * B)

        ks_c = np.zeros((2 * RPC, KP, WHROW), dtype=np.float16)
        p0 = PPC * c
        pe = min(p0 + PPC, OH * OW)
        nrow_p = (pe - p0) // OW  # full oh rows on this core (8, or 6 on core 7)
        if nrow_p:
            kblk = kernel16[p0 : p0 + nrow_p * OW]  # (nrow*62, 288, 64)
            kblk = kblk.reshape(nrow_p * 2, HOW, KH, KW * C, F)
            # -> (halfrow, kwc, ow', ch, f)
            ks_c[: nrow_p * 2, : KW * C, :] = kblk.transpose(0, 3, 1, 2, 4).reshape(
                nrow_p * 2, KW * C, WHROW
            )
            brow = np.zeros((nrow_p * 2, HOW, KH, F), dtype=np.float16)
            brow[:, :, 0, :] = bias16[p0 : p0 + nrow_p * OW].reshape(
                nrow_p * 2, HOW, F
            )
            ks_c[: nrow_p * 2, KP - 1, :] = brow.reshape(nrow_p * 2, WHROW)

        in_maps.append({"xt": xt_c, "ks": ks_c})
    return in_maps


def _run(x, kernel, bias, trace=False):
    nc = _build_program()
    in_maps = _shard_inputs(x, kernel, bias)
    res = run_bass_kernel_spmd(nc, in_maps, core_ids=list(range(NCORES)), trace=trace)
    out_full = np.empty((B, OH, OW, F), dtype=np.float32)
    for c in range(NCORES):
        rows = min(RPC, OH - RPC * c)
        o = np.asarray(res.results[c]["out"], dtype=np.float32)  # (16, 64, 496)
        # (hr, f, ow', b) -> (b, oh, half*31+ow', f)
        o = o.reshape(RPC, 2, F, HOW, B).transpose(4, 0, 1, 3, 2)
        o = o.reshape(B, RPC, OW, F)
        out_full[:, RPC * c : RPC * c + rows] = o[:, :rows]
    return out_full, res


def kernel(x, kernel, bias):
    out, _ = _run(x, kernel, bias, trace=False)
    return out


# revision 11
# speedup vs baseline: 1.1522x; 1.0049x over previous
"""LocallyConnected2D (B=16, H=W=64, C=32, 3x3 valid, F=64) on 8 trn2 cores.

out[b, oh, ow, f] = sum_{kh,kw,c} x[b, oh+kh, ow+kw, c] * kernel[p, (kh,kw,c), f] + bias[p, f]
with p = oh*62+ow.  P=3844 sharded by oh-rows across 8 cores (8 rows/core,
core 7 padded).

Per core: weights stream from HBM in fp16 as one flat [97 x 95232] tensor
(partition row 96 = bias folded into the kh=0 chunk), pulled in per-granule
DMAs on the SP queue only, so nothing ever stalls the weight stream.
Patches are pre-transposed on the host into [97 x 992] fp16 tiles (row 96 =
ones); x and output DMAs ride the Activation queue.  Each position runs 3
stationary matmuls ([97,64] weights stationary, 16 batch columns moving)
accumulating into PSUM; each granule's PSUM block is cast/copied to fp16
SBUF and written out f-major (host unscrambles).  The final granules are
small and their stores deferred so the post-stream dependency chain is
minimal.
"""

import sys

for _p in ("/opt/trn_rl_repo",):
    if _p not in sys.path:
        sys.path.insert(0, _p)

import numpy as np
from contextlib import ExitStack

import concourse.bass as bass
import concourse.bacc as bacc
import concourse.mybir as mybir
import concourse.tile as tile
from concourse.bass_utils import run_bass_kernel_spmd

F32 = mybir.dt.float32
F16 = mybir.dt.float16

B, H, W, C = 16, 64, 64, 32
KH, KW = 3, 3
OH, OW = 62, 62
F = 64
NCORES = 8
RPC = 8            # oh rows per core (core 7: 2 rows are padding)
NXR = RPC + 2      # x rows staged per core
PPC = RPC * OW     # 496 positions per core (padded for core 7)
KP = KW * C + 1    # 97 partitions: 96 contraction rows + bias/ones row
CPP = KH * F       # 192 weight columns per position

# (oh, ow0, npos) granules; last ones small to shrink the tail chain.
GRANULES = []
for _oh in range(RPC - 1):
    GRANULES.append((_oh, 0, 31))
    GRANULES.append((_oh, 31, 31))
GRANULES += [(RPC - 1, 0, 31), (RPC - 1, 31, 23), (RPC - 1, 54, 8)]
N_DEFER = 4        # store DMAs of the last N_DEFER granules issue post-stream

_cached = {}


def _build_program():
    if "nc" in _cached:
        return _cached["nc"]

    nc = bacc.Bacc(None)
    # xt[r, kw*32+c, ow*16+b] = x[b, r0+r, ow+kw, c]; row 96 = 1.0
    xt = nc.declare_dram_parameter("xt", [NXR, KP, OW * B], F16, isOutput=False)
    # ks[kw*32+c, p'*192 + ch*64 + f] = kernel[p0+p', ch*96+kw*32+c, f];
    # row 96: bias at ch==0, zero at ch 1..2
    ks = nc.declare_dram_parameter("ks", [KP, PPC * CPP], F16, isOutput=False)
    # out[f, p'*16+b]
    out = nc.declare_dram_parameter("out", [F, PPC * B], F16, isOutput=True)

    with ExitStack() as ctx:
        tc = ctx.enter_context(tile.TileContext(nc))
        tpool = ctx.enter_context(tc.tile_pool(name="tpool", bufs=NXR))
        ktpool = ctx.enter_context(tc.tile_pool(name="ktpool", bufs=4))
        pspool = ctx.enter_context(tc.tile_pool(name="pspool", bufs=4, space="PSUM"))
        stpool = ctx.enter_context(tc.tile_pool(name="stpool", bufs=N_DEFER + 4))

        T = []
        for r in range(NXR):
            t_tile = tpool.tile([KP, OW * B], F16)
            T.append(t_tile)
        for r in range(KH):
            nc.scalar.dma_start(T[r][:, :], xt[r])

        deferred = []
        seen_rows = KH
        for gi, (oh, ow0, npos) in enumerate(GRANULES):
            pbase = oh * OW + ow0
            kt = ktpool.tile([KP, 31 * CPP], F16)
            nc.sync.dma_start(
                kt[:, : npos * CPP],
                ks[:, pbase * CPP : (pbase + npos) * CPP],
            )
            if oh + KH >= seen_rows and seen_rows < NXR:
                nc.scalar.dma_start(T[seen_rows][:, :], xt[seen_rows])
                seen_rows += 1
            ps = pspool.tile([F, 31 * B], F32)
            for owl in range(npos):
                ow = ow0 + owl
                for ch in range(KH):
                    nc.tensor.matmul(
                        ps[0:F, owl * B : (owl + 1) * B],
                        kt[0:KP, (owl * KH + ch) * F : (owl * KH + ch + 1) * F],
                        T[oh + ch][0:KP, ow * B : (ow + 1) * B],
                        start=(ch == 0),
                        stop=(ch == KH - 1),
                    )
            st = stpool.tile([F, 31 * B], F16)
            nc.vector.tensor_copy(st[:, : npos * B], ps[:, : npos * B])
            dst = out[:, pbase * B : (pbase + npos) * B]
            if gi >= len(GRANULES) - N_DEFER:
                deferred.append((dst, st, npos))
            else:
                nc.scalar.dma_start(dst, st[:, : npos * B])
        for dst, st, npos in deferred:
            nc.scalar.dma_start(dst, st[:, : npos * B])

    nc.finalize()
    _cached["nc"] = nc
    return nc


def _shard_inputs(x, kernel, bias):
    x = np.asarray(x, dtype=np.float32)
    kernel = np.asarray(kernel, dtype=np.float32)
    bias = np.asarray(bias, dtype=np.float32)
    kernel16 = kernel.astype(np.float16)   # (P, 288, 64)
    bias16 = bias.astype(np.float16)       # (P, 64)
    x16 = x.astype(np.float16)             # (B, H, W, C)

    in_maps = []
    for c in range(NCORES):
        r0 = RPC * c
        nrows = min(NXR, H - r0)
        xs_c = np.zeros((NXR, B, W, C), dtype=np.float16)
        xs_c[:nrows] = np.moveaxis(x16[:, r0 : r0 + nrows], 1, 0)

        xt_c = np.empty((NXR, KP, OW * B), dtype=np.float16)
        xt_c[:, KP - 1, :] = np.float16(1.0)
        for kw in range(KW):
            # (NXR, B, OW, C) -> (NXR, C, OW, B)
            blk = xs_c[:, :, kw : kw + OW, :].transpose(0, 3, 2, 1)
            xt_c[:, kw * C : (kw + 1) * C, :] = blk.reshape(NXR, C, OW * B)

        p0 = PPC * c
        pe = min(p0 + PPC, OH * OW)
        npos = pe - p0
        kblk = np.zeros((PPC, KH, KW * C, F), dtype=np.float16)
        kblk[:npos] = kernel16[p0:pe].reshape(npos, KH, KW * C, F)
        # ks[kwc, p*192 + ch*64 + f]
        ks_c = np.zeros((KP, PPC * CPP), dtype=np.float16)
        ks_c[: KW * C] = kblk.transpose(2, 0, 1, 3).reshape(KW * C, PPC * CPP)
        brow = np.zeros((PPC, KH, F), dtype=np.float16)
        brow[:npos, 0, :] = bias16[p0:pe]
        ks_c[KP - 1] = brow.reshape(PPC * CPP)

        in_maps.append({"xt": xt_c, "ks": ks_c})
    return in_maps


def _run(x, kernel, bias, trace=False):
    nc = _build_program()
    in_maps = _shard_inputs(x, kernel, bias)
    res = run_bass_kernel_spmd(nc, in_maps, core_ids=list(range(NCORES)), trace=trace)
    out_full = np.empty((B, OH, OW, F), dtype=np.float32)
    for c in range(NCORES):
        rows = min(RPC, OH - RPC * c)
        o = np.asarray(res.results[c]["out"], dtype=np.float32)  # (64, 7936)
        # (f, p', b) -> (b, oh, ow, f)
        o = o.reshape(F, RPC, OW, B).transpose(3, 1, 2, 0)
        out_full[:, RPC * c : RPC * c + rows] = o[:, :rows]
    return out_full, res


def kernel(x, kernel, bias):
    out, _ = _run(x, kernel, bias, trace=False)
    return out
